# revision 30
# baseline (speedup 1.0000x reference)
"""Causal self-attention (B=4, T=2048, E=1024, H=16, D=64) on 8 TRN2 NeuronCores.

Sharding: core c -> batch b=c//2, head-group g=c%2 (8 heads each).

v2: fused software pipeline over 4 super-blocks (sb = 512 tokens each):
  warmup matmuls (HAM) -> A(0) B(0) -> { C(sb) attention interleaved with
  D(sb-1) output-proj and A/B(sb+1) } -> D(3).
Phase C is ScalarE(exp)-bound; interleaving keeps the PE busy with projection
work during exp stalls and keeps HAM warm. Weights resident in SBUF; x loaded
per-sb. et/vaug in bf16 (faster DVE mask, less SBUF). PSUM: 4 banks S,
2 banks AV, 2 banks shared by A/B/D.
"""
import sys

if '/opt/trn_rl_repo' not in sys.path:
    sys.path.insert(0, '/opt/trn_rl_repo')

from contextlib import ExitStack

import numpy as np
import ml_dtypes

import concourse.bass as bass
import concourse.tile as tile
from concourse import bacc, mybir
from concourse.bass_utils import run_bass_kernel_spmd

P = 128
T = 2048          # tokens per core (one batch)
E = 1024          # embed
HPC = 8           # heads per core
D = 64            # head dim
GD = HPC * D      # 512 group dims per core
NB = T // 512     # 4 super-blocks of 512 tokens
NE = E // P       # 8 contraction chunks
NT = T // P       # 16 token blocks of 128
SLOT = D + 1      # v slot width: [v(64) | ones]
SLOT8 = D + 2     # fp8 v slot width: [v(64) | ones | pad] (16B-aligned ko-stride)
F32 = mybir.dt.float32
FPR = mybir.dt.float32r
BF16 = mybir.dt.bfloat16
FP8 = mybir.dt.float8e4
EXP = mybir.ActivationFunctionType.Exp

_CACHE = {}


def _build():
    nc = bacc.Bacc("TRN2", target_bir_lowering=False, debug=False)

    xt = nc.dram_tensor("xt", [E, T], BF16, kind="ExternalInput").ap()
    wq = nc.dram_tensor("wq", [4, P, NE, P], BF16, kind="ExternalInput").ap()
    wk = nc.dram_tensor("wk", [4, P, NE, P], BF16, kind="ExternalInput").ap()
    wv = nc.dram_tensor("wv", [E, GD], BF16, kind="ExternalInput").ap()
    wp = nc.dram_tensor("wp", [GD, E], BF16, kind="ExternalInput").ap()
    bqk = nc.dram_tensor("bqk", [2, P, 4], F32, kind="ExternalInput").ap()
    bvb = nc.dram_tensor("bvb", [P, GD], BF16, kind="ExternalInput").ap()
    masks = nc.dram_tensor("masks", [P, P], BF16, kind="ExternalInput").ap()
    ones = nc.dram_tensor("ones", [P, HPC], BF16, kind="ExternalInput").ap()
    out = nc.dram_tensor("out", [T, E], F32, kind="ExternalOutput").ap()

    xtr = xt.rearrange("(eo ep) t -> ep eo t", ep=P)      # [128, 8, 2048]
    wvr = wv.rearrange("(eo ep) m -> ep eo m", ep=P)
    wpr = wp.rearrange("(co cp) o -> cp co o", cp=P)      # [128, 4, 1024]
    outr = out.rearrange("(to tp) o -> to tp o", tp=P)    # [16, 128, 1024]

    with tile.TileContext(nc) as tc, ExitStack() as ctx:
        # ---- persistent pools ----
        w_pool = ctx.enter_context(tc.tile_pool(name="wres", bufs=1))
        x_pool = ctx.enter_context(tc.tile_pool(name="xsb", bufs=2))
        qk_pool = ctx.enter_context(tc.tile_pool(name="qk", bufs=1))
        qt_pool = ctx.enter_context(tc.tile_pool(name="qt", bufs=2))
        vaug_pool = ctx.enter_context(tc.tile_pool(name="vaug", bufs=1))
        ho_pool = ctx.enter_context(tc.tile_pool(name="ho", bufs=1))
        const_pool = ctx.enter_context(tc.tile_pool(name="const", bufs=1))
        exp_pool = ctx.enter_context(tc.tile_pool(name="exps", bufs=4))
        norm_pool = ctx.enter_context(tc.tile_pool(name="norm", bufs=2))
        out_pool = ctx.enter_context(tc.tile_pool(name="outs", bufs=2))
        warm_pool = ctx.enter_context(tc.tile_pool(name="warm", bufs=1))

        # PSUM pools: S 2x[128,1024]=4 banks, AV 2x[65,512]=2 banks,
        # proj (A/B/D evac) 2x[128,512]=2 banks.
        psS = ctx.enter_context(tc.tile_pool(name="psS", bufs=2, space="PSUM"))
        psAV = ctx.enter_context(tc.tile_pool(name="psAV", bufs=1, space="PSUM"))
        psP = ctx.enter_context(tc.tile_pool(name="psP", bufs=2, space="PSUM"))

        # ---- PE warm-up: matmuls on garbage SBUF into scratch psum.
        # Depends on nothing -> runs from t~0, warms HAM while DMAs land.
        warm_src = warm_pool.tile([P, 512], BF16, tag="wsrc", name="warm_src")
        nc.gpsimd.memset(warm_src[:], 1.0)
        wps = psP.tile([P, 512], F32, tag="pp", name="warm_ps")
        for wi in range(14):
            nc.tensor.matmul(wps[:], warm_src[:, 0:P], warm_src[:],
                             start=(wi == 0), stop=(wi == 13))
        # consume so it isn't dead
        warm_sink = warm_pool.tile([P, 4], F32, tag="wsink", name="warm_sink")
        nc.vector.tensor_copy(warm_sink[:], wps[:, 0:4])

        x_sb = {}

        def emit_x_dma(sb):
            xs = x_pool.tile([P, NE, 512], BF16, tag="xs", name=f"x{sb}")
            for e2 in range(NE // 2):
                nc.sync.dma_start(xs[:, 2 * e2:2 * e2 + 2, :],
                                  xtr[:, 2 * e2:2 * e2 + 2, bass.ts(sb, 512)])
            x_sb[sb] = xs

        # x(0) first: the first A-matmuls need x chunk0 + wq chunk0; emitting
        # x before the 6MB of weights gets compute started ~4us earlier.
        emit_x_dma(0)

        # ---- resident weights (first chunks), then constants ----
        wq_t = w_pool.tile([P, NE, GD], BF16, tag="wq", name="wq_t")
        wk_t = w_pool.tile([P, NE, GD], BF16, tag="wk", name="wk_t")
        wv_t = w_pool.tile([P, NE, GD], BF16, tag="wv", name="wv_t")
        wp_t = w_pool.tile([P, 4, E], BF16, tag="wp", name="wp_t")
        # wq/wk chunked by OUTPUT dim m (host pre-transposed so each m-tile
        # is partition-major contiguous -> 2KB/partition DMA segments): the
        # first A units need only their own m-slice, so C(0,0) stops waiting
        # on the full 4MB of wq+wk.
        nc.sync.dma_start(wq_t[:, :, 0:P], wq[0])
        nc.sync.dma_start(wk_t[:, :, 0:P], wk[0])
        nc.sync.dma_start(wv_t[:, 0:2, :], wvr[:, 0:2, :])

        bqk_t = const_pool.tile([P, 2, 4], F32, tag="bqk", name="bqk_t")
        nc.sync.dma_start(bqk_t[:], bqk.rearrange("b p m -> p b m"))
        bvb_t = const_pool.tile([P, GD], BF16, tag="bvb", name="bvb_t")
        nc.sync.dma_start(bvb_t[:], bvb)
        masks_t = const_pool.tile([P, P], BF16, tag="masks", name="masks_t")
        nc.sync.dma_start(masks_t[:], masks)
        onec_t = const_pool.tile([P, HPC], BF16, tag="onec", name="onec_t")
        nc.sync.dma_start(onec_t[:], ones)

        for e2 in range(1, NE // 2):
            nc.sync.dma_start(wv_t[:, 2 * e2:2 * e2 + 2, :],
                              wvr[:, 2 * e2:2 * e2 + 2, :])
        for m in range(1, 4):
            nc.sync.dma_start(wq_t[:, :, bass.ts(m, P)], wq[m])
            nc.sync.dma_start(wk_t[:, :, bass.ts(m, P)], wk[m])
        nc.sync.dma_start(wp_t[:], wpr)

        # persistent SBUF tensors
        kT = [qk_pool.tile([P, T], BF16, tag=f"kT{i}", name=f"kT{i}") for i in range(4)]
        vaug = [vaug_pool.tile([P, HPC * SLOT], BF16, tag=f"va{t}", name=f"va{t}")
                for t in range(NT)]
        hoT = [ho_pool.tile([P, T], BF16, tag=f"hoT{i}", name=f"hoT{i}") for i in range(4)]


        def emit_A_unit(sb, qk, m, qTs):
            """q or k projection, one 128-dim tile m, feature-major out."""
            xs = x_sb[sb]
            wt = wq_t if qk == 0 else wk_t
            ps = psP.tile([P, 512], F32, tag="pp", name=f"A{sb}_{qk}_{m}")
            msl = bass.ts(m, P)
            for e in range(NE):
                nc.tensor.matmul(ps[:], wt[:, e, msl], xs[:, e, :],
                                 start=(e == 0), stop=(e == NE - 1))
            if qk == 0:
                nc.vector.tensor_scalar_add(qTs[m][:], ps[:], bqk_t[:, 0, m:m + 1])
            else:
                nc.vector.tensor_scalar_add(kT[m][:, bass.ts(sb, 512)], ps[:],
                                            bqk_t[:, 1, m:m + 1])

        def emit_B_unit(sb, tg):
            """v projection for one 128-token group, token-major + ones col."""
            xs = x_sb[sb]
            t = 4 * sb + tg
            ps = psP.tile([P, GD], F32, tag="pp", name=f"B{sb}_{tg}")
            for e in range(NE):
                nc.tensor.matmul(ps[:], xs[:, e, bass.ts(tg, P)], wv_t[:, e, :],
                                 start=(e == 0), stop=(e == NE - 1))
            nc.vector.tensor_copy(
                vaug[t][:].rearrange("p (h s) -> p h s", s=SLOT)[:, :, D:D + 1],
                onec_t[:].unsqueeze(2))
            nc.vector.tensor_tensor(
                vaug[t][:].rearrange("p (h s) -> p h s", s=SLOT)[:, :, 0:D],
                ps[:].rearrange("p (h d) -> p h d", d=D),
                bvb_t[:].rearrange("p (h d) -> p h d", d=D),
                mybir.AluOpType.add)

        def emit_D_unit(sb, t128, nh):
            """output projection for 128 tokens x 512 out-cols."""
            tsl = bass.ts(4 * sb + t128, P)
            dps = psP.tile([P, 512], F32, tag="pp", name=f"D{sb}_{t128}_{nh}")
            for c in range(4):
                nc.tensor.matmul(dps[:], hoT[c][:, tsl],
                                 wp_t[:, c, bass.ts(nh, 512)],
                                 start=(c == 0), stop=(c == 3))
            ot = out_pool.tile([P, 512], F32, tag="ot", name=f"ot{sb}_{t128}_{nh}")
            nc.vector.tensor_copy(ot[:], dps[:])
            nc.sync.dma_start(outr[4 * sb + t128, :, bass.ts(nh, 512)], ot[:])

        def emit_C_hp(qb, hp, qTs):
            """attention for head pair hp, query block qb (512 queries)."""
            slotA = bass.ds((2 * hp) * SLOT, SLOT)
            slotB = bass.ds((2 * hp + 1) * SLOT, SLOT)
            qsl0 = qb * 512
            nkb = 4 * (qb + 1)
            avA = psAV.tile([SLOT, 512], F32, tag="avA", name=f"avA{hp}_{qb}")
            avB = psAV.tile([SLOT, 512], F32, tag="avB", name=f"avB{hp}_{qb}")
            qT_m = qTs[hp]

            def emit_s(kb):
                # straddle tile j: columns q < 128j fully masked -> compute
                # only [q0:512]; triangle mask on the diagonal 128-col block.
                ksl = bass.ts(kb, P)
                j = kb - 4 * qb
                q0 = 128 * j if j > 0 else 0
                sp = psS.tile([P, 1024], F32, tag="sps", name=f"sps{hp}_{qb}_{kb}")
                et = exp_pool.tile([P, 1024], BF16, tag="expS", name=f"ex{hp}_{qb}_{kb}")
                for half, pl, ph in ((0, 0, D), (1, D, P)):
                    o = 512 * half
                    nc.tensor.matmul(sp[:, o + q0:o + 512], kT[hp][pl:ph, ksl],
                                     qT_m[pl:ph, q0:512],
                                     start=True, stop=True)
                if q0 == 0:
                    nc.scalar.activation(et[:], sp[:], EXP, scale=0.125)
                else:
                    nc.scalar.activation(
                        et[:].rearrange("p (two n) -> p two n", two=2)[:, :, q0:512],
                        sp[:].rearrange("p (two n) -> p two n", two=2)[:, :, q0:512],
                        EXP, scale=0.125)
                if j >= 0:
                    dsl = bass.ds(q0, P)
                    nc.vector.tensor_tensor(
                        et[:].rearrange("p (two n) -> p two n", two=2)[:, :, dsl],
                        et[:].rearrange("p (two n) -> p two n", two=2)[:, :, dsl],
                        masks_t[:, None, :].to_broadcast([P, 2, P]),
                        mybir.AluOpType.mult)
                return et, q0

            pend = {0: emit_s(0)}
            for kb in range(nkb):
                if kb + 1 < nkb:
                    pend[kb + 1] = emit_s(kb + 1)
                et, q0 = pend.pop(kb)
                st = (kb == 0)
                sp_ = (kb == nkb - 1)
                nc.tensor.matmul(avA[0:SLOT, q0:512], vaug[kb][:, slotA],
                                 et[:, q0:512], start=st, stop=sp_)
                nc.tensor.matmul(avB[0:SLOT, q0:512], vaug[kb][:, slotB],
                                 et[:, 512 + q0:1024], start=st, stop=sp_)

            # normalize: out[d,q]/denom[q]; denom at partition 64. Evacuate to
            # SBUF fast (frees psum), recip on [32,32] repack (low FD), gpsimd
            # partition-broadcast, multiply.
            qsl = bass.ds(qsl0, 512)
            avSA = norm_pool.tile([SLOT, 512], F32, tag="avSA", name=f"avSA{hp}_{qb}")
            nc.vector.tensor_copy(avSA[:], avA[0:SLOT, :])
            avSB = norm_pool.tile([SLOT, 512], F32, tag="avSB", name=f"avSB{hp}_{qb}")
            nc.vector.tensor_copy(avSB[:], avB[0:SLOT, :])
            d32 = norm_pool.tile([32, 32], F32, tag="d32", name=f"d32{hp}_{qb}")
            nc.sync.dma_start(d32[:, 0:16], avSA[D:SLOT, :])
            nc.sync.dma_start(d32[:, 16:32], avSB[D:SLOT, :])
            nc.vector.reciprocal(d32[:], d32[:])
            rc0A = norm_pool.tile([1, 512], F32, tag="rc0A", name=f"rc0A{hp}_{qb}")
            nc.sync.dma_start(rc0A[0:1, :], d32[:, 0:16])
            rc0B = norm_pool.tile([1, 512], F32, tag="rc0B", name=f"rc0B{hp}_{qb}")
            nc.sync.dma_start(rc0B[0:1, :], d32[:, 16:32])
            bcA = norm_pool.tile([D, 512], F32, tag="bcA", name=f"bcA{hp}_{qb}")
            nc.gpsimd.partition_broadcast(bcA[:], rc0A[0:1, :], channels=D)
            bcB = norm_pool.tile([D, 512], F32, tag="bcB", name=f"bcB{hp}_{qb}")
            nc.gpsimd.partition_broadcast(bcB[:], rc0B[0:1, :], channels=D)
            nc.vector.tensor_tensor(hoT[hp][0:D, qsl], avSA[0:D, :], bcA[:],
                                    mybir.AluOpType.mult)
            tmpB = norm_pool.tile([D, 512], BF16, tag="tmpB", name=f"tmpB{hp}_{qb}")
            nc.vector.tensor_tensor(tmpB[:], avSB[0:D, :], bcB[:],
                                    mybir.AluOpType.mult)
            nc.sync.dma_start(hoT[hp][D:P, qsl], tmpB[:])
            return avSA

        # ================= pipeline =================
        qTs_by_sb = {}

        def emit_AB(sb):
            qTs = [qt_pool.tile([P, 512], BF16, tag=f"qT{m}", name=f"qT{sb}_{m}")
                   for m in range(4)]
            qTs_by_sb[sb] = qTs
            for m in range(4):
                emit_A_unit(sb, 0, m, qTs)
            for m in range(4):
                emit_A_unit(sb, 1, m, qTs)
            for tg in range(4):
                emit_B_unit(sb, tg)

        emit_AB(0)

        for sb in range(NB):
            # filler units: projections that can absorb PE slack while
            # ScalarE grinds exp for C(sb).
            filler = []
            if sb + 1 < NB:
                filler.append(('x', sb + 1))
                qTs = [qt_pool.tile([P, 512], BF16, tag=f"qT{m}",
                                    name=f"qT{sb + 1}_{m}") for m in range(4)]
                qTs_by_sb[sb + 1] = qTs
                for m in range(4):
                    filler.append(('A', sb + 1, 0, m))
                for m in range(4):
                    filler.append(('A', sb + 1, 1, m))
                for tg in range(4):
                    filler.append(('B', sb + 1, tg))
            if sb >= 1:
                for t128 in range(4):
                    for nh in range(2):
                        filler.append(('D', sb - 1, t128, nh))

            def run_filler(frac_lo, frac_hi):
                n = len(filler)
                for u in filler[int(n * frac_lo):int(n * frac_hi)]:
                    if u[0] == 'x':
                        emit_x_dma(u[1])
                    elif u[0] == 'A':
                        emit_A_unit(u[1], u[2], u[3], qTs_by_sb[u[1]])
                    elif u[0] == 'B':
                        emit_B_unit(u[1], u[2])
                    else:
                        emit_D_unit(u[1], u[2], u[3])

            for hp in range(4):
                last_avSA = emit_C_hp(sb, hp, qTs_by_sb[sb])
                run_filler(hp / 4.0, (hp + 1) / 4.0)

        for t128 in range(4):
            for nh in range(2):
                emit_D_unit(3, t128, nh)

    nc.compile()
    return nc


def _in_maps(x, Wq, bq, Wk, bk, Wv, bv, Wp, bp):
    maskv = (np.arange(P)[:, None] <= np.arange(P)[None, :]).astype(np.float32)
    maps = []
    for c in range(8):
        b, g = divmod(c, 2)
        gs = slice(512 * g, 512 * (g + 1))
        maps.append({
            "xt": np.ascontiguousarray(x[b].T).astype(ml_dtypes.bfloat16),
            "wq": np.ascontiguousarray(
                Wq[gs, :].T.reshape(NE, P, 4, P).transpose(2, 1, 0, 3)
            ).astype(ml_dtypes.bfloat16),
            "wk": np.ascontiguousarray(
                Wk[gs, :].T.reshape(NE, P, 4, P).transpose(2, 1, 0, 3)
            ).astype(ml_dtypes.bfloat16),
            "wv": np.ascontiguousarray(Wv[gs, :].T).astype(ml_dtypes.bfloat16),
            "wp": np.ascontiguousarray(Wp[:, gs].T).astype(ml_dtypes.bfloat16),
            "bqk": np.stack([bq[gs].reshape(4, P).T, bk[gs].reshape(4, P).T]),
            "bvb": np.broadcast_to(bv[gs], (P, 512)).astype(ml_dtypes.bfloat16),
            "masks": maskv.astype(ml_dtypes.bfloat16),
            "ones": np.ones((P, HPC), ml_dtypes.bfloat16),
        })
    return maps


def kernel(x, Wq, bq, Wk, bk, Wv, bv, Wp, bp, _trace=False):
    if "nc" not in _CACHE:
        _CACHE["nc"] = _build()
    nc = _CACHE["nc"]
    res = run_bass_kernel_spmd(nc, _in_maps(x, Wq, bq, Wk, bk, Wv, bv, Wp, bp),
                               list(range(8)), trace=_trace)
    _CACHE["last_result"] = res
    out = np.empty((4, T, E), np.float32)
    for b in range(4):
        out[b] = res.results[2 * b]["out"] + res.results[2 * b + 1]["out"] + bp
    return out


# revision 32
# speedup vs baseline: 1.1864x; 1.1864x over previous
"""Causal self-attention (B=4, T=2048, E=1024, H=16, D=64) on 8 TRN2 NeuronCores.

Sharding: core c -> batch b=c//2, head-group g=c%2 (8 heads each).

v2: fused software pipeline over 4 super-blocks (sb = 512 tokens each):
  warmup matmuls (HAM) -> A(0) B(0) -> { C(sb) attention interleaved with
  D(sb-1) output-proj and A/B(sb+1) } -> D(3).
Phase C is ScalarE(exp)-bound; interleaving keeps the PE busy with projection
work during exp stalls and keeps HAM warm. Weights resident in SBUF; x loaded
per-sb. et/vaug in bf16 (faster DVE mask, less SBUF). PSUM: 4 banks S,
2 banks AV, 2 banks shared by A/B/D.
"""
import sys

if '/opt/trn_rl_repo' not in sys.path:
    sys.path.insert(0, '/opt/trn_rl_repo')

from contextlib import ExitStack

import numpy as np
import ml_dtypes

import concourse.bass as bass
import concourse.tile as tile
from concourse import bacc, mybir
from concourse.bass_utils import run_bass_kernel_spmd

P = 128
T = 2048          # tokens per core (one batch)
E = 1024          # embed
HPC = 8           # heads per core
D = 64            # head dim
GD = HPC * D      # 512 group dims per core
NB = T // 512     # 4 super-blocks of 512 tokens
NE = E // P       # 8 contraction chunks
NT = T // P       # 16 token blocks of 128
SLOT = D + 1      # v slot width: [v(64) | ones]
SLOT8 = D + 2     # fp8 v slot width: [v(64) | ones | pad] (16B-aligned ko-stride)
F32 = mybir.dt.float32
FPR = mybir.dt.float32r
BF16 = mybir.dt.bfloat16
FP8 = mybir.dt.float8e4
EXP = mybir.ActivationFunctionType.Exp

_CACHE = {}


def _build():
    nc = bacc.Bacc("TRN2", target_bir_lowering=False, debug=False)

    xt = nc.dram_tensor("xt", [E, T], BF16, kind="ExternalInput").ap()
    wq = nc.dram_tensor("wq", [E, GD], BF16, kind="ExternalInput").ap()
    wk = nc.dram_tensor("wk", [E, GD], BF16, kind="ExternalInput").ap()
    wv = nc.dram_tensor("wv", [E, GD], BF16, kind="ExternalInput").ap()
    wp = nc.dram_tensor("wp", [GD, E], BF16, kind="ExternalInput").ap()
    bqk = nc.dram_tensor("bqk", [2, P, 4], F32, kind="ExternalInput").ap()
    bvb = nc.dram_tensor("bvb", [P, GD], BF16, kind="ExternalInput").ap()
    masks = nc.dram_tensor("masks", [P, P], BF16, kind="ExternalInput").ap()
    ones = nc.dram_tensor("ones", [P, HPC], BF16, kind="ExternalInput").ap()
    out = nc.dram_tensor("out", [T, E], F32, kind="ExternalOutput").ap()

    xtr = xt.rearrange("(eo ep) t -> ep eo t", ep=P)      # [128, 8, 2048]
    wqr = wq.rearrange("(eo ep) m -> ep eo m", ep=P)      # [128, 8, 512]
    wkr = wk.rearrange("(eo ep) m -> ep eo m", ep=P)
    wvr = wv.rearrange("(eo ep) m -> ep eo m", ep=P)
    wpr = wp.rearrange("(co cp) o -> cp co o", cp=P)      # [128, 4, 1024]
    outr = out.rearrange("(to tp) o -> to tp o", tp=P)    # [16, 128, 1024]

    with tile.TileContext(nc) as tc, ExitStack() as ctx:
        # ---- persistent pools ----
        w_pool = ctx.enter_context(tc.tile_pool(name="wres", bufs=1))
        x_pool = ctx.enter_context(tc.tile_pool(name="xsb", bufs=2))
        qk_pool = ctx.enter_context(tc.tile_pool(name="qk", bufs=1))
        qt_pool = ctx.enter_context(tc.tile_pool(name="qt", bufs=2))
        vaug_pool = ctx.enter_context(tc.tile_pool(name="vaug", bufs=1))
        ho_pool = ctx.enter_context(tc.tile_pool(name="ho", bufs=1))
        const_pool = ctx.enter_context(tc.tile_pool(name="const", bufs=1))
        exp_pool = ctx.enter_context(tc.tile_pool(name="exps", bufs=4))
        norm_pool = ctx.enter_context(tc.tile_pool(name="norm", bufs=2))
        out_pool = ctx.enter_context(tc.tile_pool(name="outs", bufs=2))
        warm_pool = ctx.enter_context(tc.tile_pool(name="warm", bufs=1))

        # PSUM pools: S 2x[128,1024]=4 banks, AV 2x[65,512]=2 banks,
        # proj (A/B/D evac) 2x[128,512]=2 banks.
        psS = ctx.enter_context(tc.tile_pool(name="psS", bufs=2, space="PSUM"))
        psAV = ctx.enter_context(tc.tile_pool(name="psAV", bufs=1, space="PSUM"))
        psP = ctx.enter_context(tc.tile_pool(name="psP", bufs=2, space="PSUM"))

        # ---- PE warm-up: matmuls on garbage SBUF into scratch psum.
        # Depends on nothing -> runs from t~0, warms HAM while DMAs land.
        warm_src = warm_pool.tile([P, 512], BF16, tag="wsrc", name="warm_src")
        nc.gpsimd.memset(warm_src[:], 1.0)
        wps = psP.tile([P, 512], F32, tag="pp", name="warm_ps")
        for wi in range(14):
            nc.tensor.matmul(wps[:], warm_src[:, 0:P], warm_src[:],
                             start=(wi == 0), stop=(wi == 13))
        # consume so it isn't dead
        warm_sink = warm_pool.tile([P, 4], F32, tag="wsink", name="warm_sink")
        nc.vector.tensor_copy(warm_sink[:], wps[:, 0:4])

        x_sb = {}

        def emit_x_dma(sb):
            xs = x_pool.tile([P, NE, 512], BF16, tag="xs", name=f"x{sb}")
            for e2 in range(NE // 2):
                nc.sync.dma_start(xs[:, 2 * e2:2 * e2 + 2, :],
                                  xtr[:, 2 * e2:2 * e2 + 2, bass.ts(sb, 512)])
            x_sb[sb] = xs

        # x(0) first: the first A-matmuls need x chunk0 + wq chunk0; emitting
        # x before the 6MB of weights gets compute started ~4us earlier.
        emit_x_dma(0)

        # ---- resident weights (first chunks), then constants ----
        wq_t = w_pool.tile([P, NE, GD], BF16, tag="wq", name="wq_t")
        wk_t = w_pool.tile([P, NE, GD], BF16, tag="wk", name="wk_t")
        wv_t = w_pool.tile([P, NE, GD], BF16, tag="wv", name="wv_t")
        wp_t = w_pool.tile([P, 4, E], BF16, tag="wp", name="wp_t")
        # chunked weight DMAs so the first A-matmuls can start early
        nc.sync.dma_start(wq_t[:, 0:2, :], wqr[:, 0:2, :])
        nc.sync.dma_start(wk_t[:, 0:2, :], wkr[:, 0:2, :])

        bqk_t = const_pool.tile([P, 2, 4], F32, tag="bqk", name="bqk_t")
        nc.sync.dma_start(bqk_t[:], bqk.rearrange("b p m -> p b m"))
        bvb_t = const_pool.tile([P, GD], BF16, tag="bvb", name="bvb_t")
        nc.sync.dma_start(bvb_t[:], bvb)
        masks_t = const_pool.tile([P, P], BF16, tag="masks", name="masks_t")
        nc.sync.dma_start(masks_t[:], masks)
        onec_t = const_pool.tile([P, HPC], BF16, tag="onec", name="onec_t")
        nc.sync.dma_start(onec_t[:], ones)

        # wv before the wq/wk remainder: all four B(0) units need all of wv
        # and serialize ~5us of PE work after its last byte, while the A units
        # pipeline per-chunk with wq/wk arrival -- so wv must not land last.
        for e2 in range(NE // 2):
            nc.sync.dma_start(wv_t[:, 2 * e2:2 * e2 + 2, :],
                              wvr[:, 2 * e2:2 * e2 + 2, :])
        for e2 in range(1, NE // 2):
            nc.sync.dma_start(wq_t[:, 2 * e2:2 * e2 + 2, :],
                              wqr[:, 2 * e2:2 * e2 + 2, :])
            nc.sync.dma_start(wk_t[:, 2 * e2:2 * e2 + 2, :],
                              wkr[:, 2 * e2:2 * e2 + 2, :])
        nc.sync.dma_start(wp_t[:], wpr)

        # persistent SBUF tensors
        kT = [qk_pool.tile([P, T], BF16, tag=f"kT{i}", name=f"kT{i}") for i in range(4)]
        vaug = [vaug_pool.tile([P, HPC * SLOT], BF16, tag=f"va{t}", name=f"va{t}")
                for t in range(NT)]
        hoT = [ho_pool.tile([P, T], BF16, tag=f"hoT{i}", name=f"hoT{i}") for i in range(4)]


        def emit_A_unit(sb, qk, m, qTs):
            """q or k projection, one 128-dim tile m, feature-major out."""
            xs = x_sb[sb]
            wt = wq_t if qk == 0 else wk_t
            ps = psP.tile([P, 512], F32, tag="pp", name=f"A{sb}_{qk}_{m}")
            msl = bass.ts(m, P)
            for e in range(NE):
                nc.tensor.matmul(ps[:], wt[:, e, msl], xs[:, e, :],
                                 start=(e == 0), stop=(e == NE - 1))
            if qk == 0:
                nc.vector.tensor_scalar_add(qTs[m][:], ps[:], bqk_t[:, 0, m:m + 1])
            else:
                nc.vector.tensor_scalar_add(kT[m][:, bass.ts(sb, 512)], ps[:],
                                            bqk_t[:, 1, m:m + 1])

        def emit_B_unit(sb, tg):
            """v projection for one 128-token group, token-major + ones col."""
            xs = x_sb[sb]
            t = 4 * sb + tg
            ps = psP.tile([P, GD], F32, tag="pp", name=f"B{sb}_{tg}")
            for e in range(NE):
                nc.tensor.matmul(ps[:], xs[:, e, bass.ts(tg, P)], wv_t[:, e, :],
                                 start=(e == 0), stop=(e == NE - 1))
            nc.vector.tensor_copy(
                vaug[t][:].rearrange("p (h s) -> p h s", s=SLOT)[:, :, D:D + 1],
                onec_t[:].unsqueeze(2))
            nc.vector.tensor_tensor(
                vaug[t][:].rearrange("p (h s) -> p h s", s=SLOT)[:, :, 0:D],
                ps[:].rearrange("p (h d) -> p h d", d=D),
                bvb_t[:].rearrange("p (h d) -> p h d", d=D),
                mybir.AluOpType.add)

        def emit_D_unit(sb, t128, nh):
            """output projection for 128 tokens x 512 out-cols."""
            tsl = bass.ts(4 * sb + t128, P)
            dps = psP.tile([P, 512], F32, tag="pp", name=f"D{sb}_{t128}_{nh}")
            for c in range(4):
                nc.tensor.matmul(dps[:], hoT[c][:, tsl],
                                 wp_t[:, c, bass.ts(nh, 512)],
                                 start=(c == 0), stop=(c == 3))
            ot = out_pool.tile([P, 512], F32, tag="ot", name=f"ot{sb}_{t128}_{nh}")
            nc.vector.tensor_copy(ot[:], dps[:])
            nc.sync.dma_start(outr[4 * sb + t128, :, bass.ts(nh, 512)], ot[:])

        def emit_C_hp(qb, hp, qTs):
            """attention for head pair hp, query block qb (512 queries)."""
            slotA = bass.ds((2 * hp) * SLOT, SLOT)
            slotB = bass.ds((2 * hp + 1) * SLOT, SLOT)
            qsl0 = qb * 512
            nkb = 4 * (qb + 1)
            avA = psAV.tile([SLOT, 512], F32, tag="avA", name=f"avA{hp}_{qb}")
            avB = psAV.tile([SLOT, 512], F32, tag="avB", name=f"avB{hp}_{qb}")
            qT_m = qTs[hp]

            def emit_s(kb):
                # straddle tile j: columns q < 128j fully masked -> compute
                # only [q0:512]; triangle mask on the diagonal 128-col block.
                ksl = bass.ts(kb, P)
                j = kb - 4 * qb
                q0 = 128 * j if j > 0 else 0
                sp = psS.tile([P, 1024], F32, tag="sps", name=f"sps{hp}_{qb}_{kb}")
                et = exp_pool.tile([P, 1024], BF16, tag="expS", name=f"ex{hp}_{qb}_{kb}")
                for half, pl, ph in ((0, 0, D), (1, D, P)):
                    o = 512 * half
                    nc.tensor.matmul(sp[:, o + q0:o + 512], kT[hp][pl:ph, ksl],
                                     qT_m[pl:ph, q0:512],
                                     start=True, stop=True)
                if q0 == 0:
                    nc.scalar.activation(et[:], sp[:], EXP, scale=0.125)
                else:
                    nc.scalar.activation(
                        et[:].rearrange("p (two n) -> p two n", two=2)[:, :, q0:512],
                        sp[:].rearrange("p (two n) -> p two n", two=2)[:, :, q0:512],
                        EXP, scale=0.125)
                if j >= 0:
                    dsl = bass.ds(q0, P)
                    nc.vector.tensor_tensor(
                        et[:].rearrange("p (two n) -> p two n", two=2)[:, :, dsl],
                        et[:].rearrange("p (two n) -> p two n", two=2)[:, :, dsl],
                        masks_t[:, None, :].to_broadcast([P, 2, P]),
                        mybir.AluOpType.mult)
                return et, q0

            pend = {0: emit_s(0)}
            for kb in range(nkb):
                if kb + 1 < nkb:
                    pend[kb + 1] = emit_s(kb + 1)
                et, q0 = pend.pop(kb)
                st = (kb == 0)
                sp_ = (kb == nkb - 1)
                nc.tensor.matmul(avA[0:SLOT, q0:512], vaug[kb][:, slotA],
                                 et[:, q0:512], start=st, stop=sp_)
                nc.tensor.matmul(avB[0:SLOT, q0:512], vaug[kb][:, slotB],
                                 et[:, 512 + q0:1024], start=st, stop=sp_)

            # normalize: out[d,q]/denom[q]; denom at partition 64. Evacuate to
            # SBUF fast (frees psum), recip on [32,32] repack (low FD), gpsimd
            # partition-broadcast, multiply.
            qsl = bass.ds(qsl0, 512)
            avSA = norm_pool.tile([SLOT, 512], F32, tag="avSA", name=f"avSA{hp}_{qb}")
            nc.vector.tensor_copy(avSA[:], avA[0:SLOT, :])
            avSB = norm_pool.tile([SLOT, 512], F32, tag="avSB", name=f"avSB{hp}_{qb}")
            nc.vector.tensor_copy(avSB[:], avB[0:SLOT, :])
            d32 = norm_pool.tile([32, 32], F32, tag="d32", name=f"d32{hp}_{qb}")
            nc.sync.dma_start(d32[:, 0:16], avSA[D:SLOT, :])
            nc.sync.dma_start(d32[:, 16:32], avSB[D:SLOT, :])
            nc.vector.reciprocal(d32[:], d32[:])
            rc0A = norm_pool.tile([1, 512], F32, tag="rc0A", name=f"rc0A{hp}_{qb}")
            nc.sync.dma_start(rc0A[0:1, :], d32[:, 0:16])
            rc0B = norm_pool.tile([1, 512], F32, tag="rc0B", name=f"rc0B{hp}_{qb}")
            nc.sync.dma_start(rc0B[0:1, :], d32[:, 16:32])
            bcA = norm_pool.tile([D, 512], F32, tag="bcA", name=f"bcA{hp}_{qb}")
            nc.gpsimd.partition_broadcast(bcA[:], rc0A[0:1, :], channels=D)
            bcB = norm_pool.tile([D, 512], F32, tag="bcB", name=f"bcB{hp}_{qb}")
            nc.gpsimd.partition_broadcast(bcB[:], rc0B[0:1, :], channels=D)
            nc.vector.tensor_tensor(hoT[hp][0:D, qsl], avSA[0:D, :], bcA[:],
                                    mybir.AluOpType.mult)
            tmpB = norm_pool.tile([D, 512], BF16, tag="tmpB", name=f"tmpB{hp}_{qb}")
            nc.vector.tensor_tensor(tmpB[:], avSB[0:D, :], bcB[:],
                                    mybir.AluOpType.mult)
            nc.sync.dma_start(hoT[hp][D:P, qsl], tmpB[:])
            return avSA

        # ================= pipeline =================
        qTs_by_sb = {}

        def emit_AB(sb):
            qTs = [qt_pool.tile([P, 512], BF16, tag=f"qT{m}", name=f"qT{sb}_{m}")
                   for m in range(4)]
            qTs_by_sb[sb] = qTs
            for m in range(4):
                emit_A_unit(sb, 0, m, qTs)
            for m in range(4):
                emit_A_unit(sb, 1, m, qTs)
            for tg in range(4):
                emit_B_unit(sb, tg)

        emit_AB(0)

        for sb in range(NB):
            # filler units: projections that can absorb PE slack while
            # ScalarE grinds exp for C(sb).
            filler = []
            if sb + 1 < NB:
                filler.append(('x', sb + 1))
                qTs = [qt_pool.tile([P, 512], BF16, tag=f"qT{m}",
                                    name=f"qT{sb + 1}_{m}") for m in range(4)]
                qTs_by_sb[sb + 1] = qTs
                for m in range(4):
                    filler.append(('A', sb + 1, 0, m))
                for m in range(4):
                    filler.append(('A', sb + 1, 1, m))
                for tg in range(4):
                    filler.append(('B', sb + 1, tg))
            if sb >= 1:
                for t128 in range(4):
                    for nh in range(2):
                        filler.append(('D', sb - 1, t128, nh))

            def run_filler(frac_lo, frac_hi):
                n = len(filler)
                for u in filler[int(n * frac_lo):int(n * frac_hi)]:
                    if u[0] == 'x':
                        emit_x_dma(u[1])
                    elif u[0] == 'A':
                        emit_A_unit(u[1], u[2], u[3], qTs_by_sb[u[1]])
                    elif u[0] == 'B':
                        emit_B_unit(u[1], u[2])
                    else:
                        emit_D_unit(u[1], u[2], u[3])

            for hp in range(4):
                last_avSA = emit_C_hp(sb, hp, qTs_by_sb[sb])
                run_filler(hp / 4.0, (hp + 1) / 4.0)

        for t128 in range(4):
            for nh in range(2):
                emit_D_unit(3, t128, nh)

    nc.compile()
    return nc


def _in_maps(x, Wq, bq, Wk, bk, Wv, bv, Wp, bp):
    maskv = (np.arange(P)[:, None] <= np.arange(P)[None, :]).astype(np.float32)
    maps = []
    for c in range(8):
        b, g = divmod(c, 2)
        gs = slice(512 * g, 512 * (g + 1))
        maps.append({
            "xt": np.ascontiguousarray(x[b].T).astype(ml_dtypes.bfloat16),
            "wq": np.ascontiguousarray(Wq[gs, :].T).astype(ml_dtypes.bfloat16),
            "wk": np.ascontiguousarray(Wk[gs, :].T).astype(ml_dtypes.bfloat16),
            "wv": np.ascontiguousarray(Wv[gs, :].T).astype(ml_dtypes.bfloat16),
            "wp": np.ascontiguousarray(Wp[:, gs].T).astype(ml_dtypes.bfloat16),
            "bqk": np.stack([bq[gs].reshape(4, P).T, bk[gs].reshape(4, P).T]),
            "bvb": np.broadcast_to(bv[gs], (P, 512)).astype(ml_dtypes.bfloat16),
            "masks": maskv.astype(ml_dtypes.bfloat16),
            "ones": np.ones((P, HPC), ml_dtypes.bfloat16),
        })
    return maps


def kernel(x, Wq, bq, Wk, bk, Wv, bv, Wp, bp, _trace=False):
    if "nc" not in _CACHE:
        _CACHE["nc"] = _build()
    nc = _CACHE["nc"]
    res = run_bass_kernel_spmd(nc, _in_maps(x, Wq, bq, Wk, bk, Wv, bv, Wp, bp),
                               list(range(8)), trace=_trace)
    _CACHE["last_result"] = res
    out = np.empty((4, T, E), np.float32)
    for b in range(4):
        out[b] = res.results[2 * b]["out"] + res.results[2 * b + 1]["out"] + bp
    return out


# revision 33
# speedup vs baseline: 1.2072x; 1.0175x over previous
"""Causal self-attention (B=4, T=2048, E=1024, H=16, D=64) on 8 TRN2 NeuronCores.

Sharding: core c -> batch b=c//2, head-group g=c%2 (8 heads each).

v2: fused software pipeline over 4 super-blocks (sb = 512 tokens each):
  warmup matmuls (HAM) -> A(0) B(0) -> { C(sb) attention interleaved with
  D(sb-1) output-proj and A/B(sb+1) } -> D(3).
Phase C is ScalarE(exp)-bound; interleaving keeps the PE busy with projection
work during exp stalls and keeps HAM warm. Weights resident in SBUF; x loaded
per-sb. et/vaug in bf16 (faster DVE mask, less SBUF). PSUM: 4 banks S,
2 banks AV, 2 banks shared by A/B/D.
"""
import sys

if '/opt/trn_rl_repo' not in sys.path:
    sys.path.insert(0, '/opt/trn_rl_repo')

from contextlib import ExitStack

import numpy as np
import ml_dtypes

import concourse.bass as bass
import concourse.tile as tile
from concourse import bacc, mybir
from concourse.bass_utils import run_bass_kernel_spmd

P = 128
T = 2048          # tokens per core (one batch)
E = 1024          # embed
HPC = 8           # heads per core
D = 64            # head dim
GD = HPC * D      # 512 group dims per core
NB = T // 512     # 4 super-blocks of 512 tokens
NE = E // P       # 8 contraction chunks
NT = T // P       # 16 token blocks of 128
SLOT = D + 1      # v slot width: [v(64) | ones]
SLOT8 = D + 2     # fp8 v slot width: [v(64) | ones | pad] (16B-aligned ko-stride)
F32 = mybir.dt.float32
FPR = mybir.dt.float32r
BF16 = mybir.dt.bfloat16
FP8 = mybir.dt.float8e4
EXP = mybir.ActivationFunctionType.Exp

_CACHE = {}


def _build():
    nc = bacc.Bacc("TRN2", target_bir_lowering=False, debug=False)

    xt = nc.dram_tensor("xt", [E, T], BF16, kind="ExternalInput").ap()
    wq = nc.dram_tensor("wq", [E, GD], BF16, kind="ExternalInput").ap()
    wk = nc.dram_tensor("wk", [E, GD], BF16, kind="ExternalInput").ap()
    wv = nc.dram_tensor("wv", [E, GD], BF16, kind="ExternalInput").ap()
    wp = nc.dram_tensor("wp", [GD, E], BF16, kind="ExternalInput").ap()
    bqk = nc.dram_tensor("bqk", [2, P, 4], F32, kind="ExternalInput").ap()
    bvb = nc.dram_tensor("bvb", [P, GD], BF16, kind="ExternalInput").ap()
    masks = nc.dram_tensor("masks", [P, P], BF16, kind="ExternalInput").ap()
    ones = nc.dram_tensor("ones", [P, HPC], BF16, kind="ExternalInput").ap()
    out = nc.dram_tensor("out", [T, E], F32, kind="ExternalOutput").ap()

    xtr = xt.rearrange("(eo ep) t -> ep eo t", ep=P)      # [128, 8, 2048]
    wqr = wq.rearrange("(eo ep) m -> ep eo m", ep=P)      # [128, 8, 512]
    wkr = wk.rearrange("(eo ep) m -> ep eo m", ep=P)
    wvr = wv.rearrange("(eo ep) m -> ep eo m", ep=P)
    wpr = wp.rearrange("(co cp) o -> cp co o", cp=P)      # [128, 4, 1024]
    outr = out.rearrange("(to tp) o -> to tp o", tp=P)    # [16, 128, 1024]

    with tile.TileContext(nc) as tc, ExitStack() as ctx:
        # ---- persistent pools ----
        w_pool = ctx.enter_context(tc.tile_pool(name="wres", bufs=1))
        x_pool = ctx.enter_context(tc.tile_pool(name="xsb", bufs=2))
        qk_pool = ctx.enter_context(tc.tile_pool(name="qk", bufs=1))
        qt_pool = ctx.enter_context(tc.tile_pool(name="qt", bufs=2))
        vaug_pool = ctx.enter_context(tc.tile_pool(name="vaug", bufs=1))
        ho_pool = ctx.enter_context(tc.tile_pool(name="ho", bufs=1))
        const_pool = ctx.enter_context(tc.tile_pool(name="const", bufs=1))
        exp_pool = ctx.enter_context(tc.tile_pool(name="exps", bufs=4))
        norm_pool = ctx.enter_context(tc.tile_pool(name="norm", bufs=2))
        out_pool = ctx.enter_context(tc.tile_pool(name="outs", bufs=2))
        warm_pool = ctx.enter_context(tc.tile_pool(name="warm", bufs=1))

        # PSUM pools: S 2x[128,1024]=4 banks, AV 2x[65,512]=2 banks,
        # proj (A/B/D evac) 2x[128,512]=2 banks.
        psS = ctx.enter_context(tc.tile_pool(name="psS", bufs=2, space="PSUM"))
        psAV = ctx.enter_context(tc.tile_pool(name="psAV", bufs=1, space="PSUM"))
        psP = ctx.enter_context(tc.tile_pool(name="psP", bufs=2, space="PSUM"))

        # ---- PE warm-up: matmuls on garbage SBUF into scratch psum.
        # Depends on nothing -> runs from t~0, warms HAM while DMAs land.
        warm_src = warm_pool.tile([P, 512], BF16, tag="wsrc", name="warm_src")
        nc.gpsimd.memset(warm_src[:], 1.0)
        wps = psP.tile([P, 512], F32, tag="pp", name="warm_ps")
        for wi in range(14):
            nc.tensor.matmul(wps[:], warm_src[:, 0:P], warm_src[:],
                             start=(wi == 0), stop=(wi == 13))
        # consume so it isn't dead
        warm_sink = warm_pool.tile([P, 4], F32, tag="wsink", name="warm_sink")
        nc.vector.tensor_copy(warm_sink[:], wps[:, 0:4])

        x_sb = {}

        def emit_x_dma(sb):
            xs = x_pool.tile([P, NE, 512], BF16, tag="xs", name=f"x{sb}")
            for e2 in range(NE // 2):
                nc.sync.dma_start(xs[:, 2 * e2:2 * e2 + 2, :],
                                  xtr[:, 2 * e2:2 * e2 + 2, bass.ts(sb, 512)])
            x_sb[sb] = xs

        # x(0) first: the first A-matmuls need x chunk0 + wq chunk0; emitting
        # x before the 6MB of weights gets compute started ~4us earlier.
        emit_x_dma(0)

        # ---- resident weights (first chunks), then constants ----
        wq_t = w_pool.tile([P, NE, GD], BF16, tag="wq", name="wq_t")
        wk_t = w_pool.tile([P, NE, GD], BF16, tag="wk", name="wk_t")
        wv_t = w_pool.tile([P, NE, GD], BF16, tag="wv", name="wv_t")
        wp_t = w_pool.tile([P, 4, E], BF16, tag="wp", name="wp_t")
        # chunked weight DMAs so the first A-matmuls can start early
        nc.sync.dma_start(wq_t[:, 0:2, :], wqr[:, 0:2, :])
        nc.sync.dma_start(wk_t[:, 0:2, :], wkr[:, 0:2, :])

        bqk_t = const_pool.tile([P, 2, 4], F32, tag="bqk", name="bqk_t")
        nc.sync.dma_start(bqk_t[:], bqk.rearrange("b p m -> p b m"))
        bvb_t = const_pool.tile([P, GD], BF16, tag="bvb", name="bvb_t")
        nc.sync.dma_start(bvb_t[:], bvb)
        masks_t = const_pool.tile([P, P], BF16, tag="masks", name="masks_t")
        nc.sync.dma_start(masks_t[:], masks)
        onec_t = const_pool.tile([P, HPC], BF16, tag="onec", name="onec_t")
        nc.sync.dma_start(onec_t[:], ones)

        for e2 in range(1, NE // 2):
            nc.sync.dma_start(wq_t[:, 2 * e2:2 * e2 + 2, :],
                              wqr[:, 2 * e2:2 * e2 + 2, :])
            nc.sync.dma_start(wk_t[:, 2 * e2:2 * e2 + 2, :],
                              wkr[:, 2 * e2:2 * e2 + 2, :])
        for e2 in range(NE // 2):
            nc.sync.dma_start(wv_t[:, 2 * e2:2 * e2 + 2, :],
                              wvr[:, 2 * e2:2 * e2 + 2, :])
        nc.sync.dma_start(wp_t[:], wpr)

        # persistent SBUF tensors
        kT = [qk_pool.tile([P, T], BF16, tag=f"kT{i}", name=f"kT{i}") for i in range(4)]
        vaug = [vaug_pool.tile([P, HPC * SLOT], BF16, tag=f"va{t}", name=f"va{t}")
                for t in range(NT)]
        hoT = [ho_pool.tile([P, T], BF16, tag=f"hoT{i}", name=f"hoT{i}") for i in range(4)]


        def emit_A_unit(sb, qk, m, qTs):
            """q or k projection, one 128-dim tile m, feature-major out."""
            xs = x_sb[sb]
            wt = wq_t if qk == 0 else wk_t
            ps = psP.tile([P, 512], F32, tag="pp", name=f"A{sb}_{qk}_{m}")
            msl = bass.ts(m, P)
            for e in range(NE):
                nc.tensor.matmul(ps[:], wt[:, e, msl], xs[:, e, :],
                                 start=(e == 0), stop=(e == NE - 1))
            if qk == 0:
                nc.vector.tensor_scalar_add(qTs[m][:], ps[:], bqk_t[:, 0, m:m + 1])
            else:
                nc.vector.tensor_scalar_add(kT[m][:, bass.ts(sb, 512)], ps[:],
                                            bqk_t[:, 1, m:m + 1])

        def emit_B_unit(sb, tg):
            """v projection for one 128-token group, token-major + ones col."""
            xs = x_sb[sb]
            t = 4 * sb + tg
            ps = psP.tile([P, GD], F32, tag="pp", name=f"B{sb}_{tg}")
            for e in range(NE):
                nc.tensor.matmul(ps[:], xs[:, e, bass.ts(tg, P)], wv_t[:, e, :],
                                 start=(e == 0), stop=(e == NE - 1))
            nc.vector.tensor_copy(
                vaug[t][:].rearrange("p (h s) -> p h s", s=SLOT)[:, :, D:D + 1],
                onec_t[:].unsqueeze(2))
            nc.vector.tensor_tensor(
                vaug[t][:].rearrange("p (h s) -> p h s", s=SLOT)[:, :, 0:D],
                ps[:].rearrange("p (h d) -> p h d", d=D),
                bvb_t[:].rearrange("p (h d) -> p h d", d=D),
                mybir.AluOpType.add)

        def emit_D_unit(sb, t128, nh):
            """output projection for 128 tokens x 512 out-cols."""
            tsl = bass.ts(4 * sb + t128, P)
            dps = psP.tile([P, 512], F32, tag="pp", name=f"D{sb}_{t128}_{nh}")
            for c in range(4):
                nc.tensor.matmul(dps[:], hoT[c][:, tsl],
                                 wp_t[:, c, bass.ts(nh, 512)],
                                 start=(c == 0), stop=(c == 3))
            ot = out_pool.tile([P, 512], F32, tag="ot", name=f"ot{sb}_{t128}_{nh}")
            nc.vector.tensor_copy(ot[:], dps[:])
            nc.sync.dma_start(outr[4 * sb + t128, :, bass.ts(nh, 512)], ot[:])

        def emit_C_hp(qb, hp, qTs):
            """attention for head pair hp, query block qb (512 queries)."""
            slotA = bass.ds((2 * hp) * SLOT, SLOT)
            slotB = bass.ds((2 * hp + 1) * SLOT, SLOT)
            qsl0 = qb * 512
            nkb = 4 * (qb + 1)
            avA = psAV.tile([SLOT, 512], F32, tag="avA", name=f"avA{hp}_{qb}")
            avB = psAV.tile([SLOT, 512], F32, tag="avB", name=f"avB{hp}_{qb}")
            qT_m = qTs[hp]

            def emit_s(kb):
                # straddle tile j: columns q < 128j fully masked -> compute
                # only [q0:512]; triangle mask on the diagonal 128-col block.
                ksl = bass.ts(kb, P)
                j = kb - 4 * qb
                q0 = 128 * j if j > 0 else 0
                sp = psS.tile([P, 1024], F32, tag="sps", name=f"sps{hp}_{qb}_{kb}")
                et = exp_pool.tile([P, 1024], BF16, tag="expS", name=f"ex{hp}_{qb}_{kb}")
                for half, pl, ph in ((0, 0, D), (1, D, P)):
                    o = 512 * half
                    nc.tensor.matmul(sp[:, o + q0:o + 512], kT[hp][pl:ph, ksl],
                                     qT_m[pl:ph, q0:512],
                                     start=True, stop=True)
                if q0 == 0:
                    nc.scalar.activation(et[:], sp[:], EXP, scale=0.125)
                else:
                    nc.scalar.activation(
                        et[:].rearrange("p (two n) -> p two n", two=2)[:, :, q0:512],
                        sp[:].rearrange("p (two n) -> p two n", two=2)[:, :, q0:512],
                        EXP, scale=0.125)
                if j >= 0:
                    dsl = bass.ds(q0, P)
                    nc.vector.tensor_tensor(
                        et[:].rearrange("p (two n) -> p two n", two=2)[:, :, dsl],
                        et[:].rearrange("p (two n) -> p two n", two=2)[:, :, dsl],
                        masks_t[:, None, :].to_broadcast([P, 2, P]),
                        mybir.AluOpType.mult)
                return et, q0

            pend = {0: emit_s(0)}
            for kb in range(nkb):
                if kb + 1 < nkb:
                    pend[kb + 1] = emit_s(kb + 1)
                et, q0 = pend.pop(kb)
                st = (kb == 0)
                sp_ = (kb == nkb - 1)
                nc.tensor.matmul(avA[0:SLOT, q0:512], vaug[kb][:, slotA],
                                 et[:, q0:512], start=st, stop=sp_)
                nc.tensor.matmul(avB[0:SLOT, q0:512], vaug[kb][:, slotB],
                                 et[:, 512 + q0:1024], start=st, stop=sp_)

            # normalize: out[d,q]/denom[q]; denom at partition 64. Evacuate to
            # SBUF fast (frees psum), recip on [32,32] repack (low FD), gpsimd
            # partition-broadcast, multiply.
            qsl = bass.ds(qsl0, 512)
            avSA = norm_pool.tile([SLOT, 512], F32, tag="avSA", name=f"avSA{hp}_{qb}")
            nc.vector.tensor_copy(avSA[:], avA[0:SLOT, :])
            avSB = norm_pool.tile([SLOT, 512], F32, tag="avSB", name=f"avSB{hp}_{qb}")
            nc.vector.tensor_copy(avSB[:], avB[0:SLOT, :])
            d32 = norm_pool.tile([32, 32], F32, tag="d32", name=f"d32{hp}_{qb}")
            nc.sync.dma_start(d32[:, 0:16], avSA[D:SLOT, :])
            nc.sync.dma_start(d32[:, 16:32], avSB[D:SLOT, :])
            nc.vector.reciprocal(d32[:], d32[:])
            rc0A = norm_pool.tile([1, 512], F32, tag="rc0A", name=f"rc0A{hp}_{qb}")
            nc.sync.dma_start(rc0A[0:1, :], d32[:, 0:16])
            rc0B = norm_pool.tile([1, 512], F32, tag="rc0B", name=f"rc0B{hp}_{qb}")
            nc.sync.dma_start(rc0B[0:1, :], d32[:, 16:32])
            bcA = norm_pool.tile([D, 512], F32, tag="bcA", name=f"bcA{hp}_{qb}")
            nc.gpsimd.partition_broadcast(bcA[:], rc0A[0:1, :], channels=D)
            bcB = norm_pool.tile([D, 512], F32, tag="bcB", name=f"bcB{hp}_{qb}")
            nc.gpsimd.partition_broadcast(bcB[:], rc0B[0:1, :], channels=D)
            nc.vector.tensor_tensor(hoT[hp][0:D, qsl], avSA[0:D, :], bcA[:],
                                    mybir.AluOpType.mult)
            tmpB = norm_pool.tile([D, 512], BF16, tag="tmpB", name=f"tmpB{hp}_{qb}")
            nc.vector.tensor_tensor(tmpB[:], avSB[0:D, :], bcB[:],
                                    mybir.AluOpType.mult)
            nc.sync.dma_start(hoT[hp][D:P, qsl], tmpB[:])
            return avSA

        # ================= pipeline =================
        qTs_by_sb = {}

        def emit_AB(sb):
            qTs = [qt_pool.tile([P, 512], BF16, tag=f"qT{m}", name=f"qT{sb}_{m}")
                   for m in range(4)]
            qTs_by_sb[sb] = qTs
            for m in range(4):
                emit_A_unit(sb, 0, m, qTs)
            for m in range(4):
                emit_A_unit(sb, 1, m, qTs)
            for tg in range(4):
                emit_B_unit(sb, tg)

        emit_AB(0)

        for sb in range(NB):
            # filler units: projections that can absorb PE slack while
            # ScalarE grinds exp for C(sb).
            filler = []
            if sb + 1 < NB:
                filler.append(('x', sb + 1))
                qTs = [qt_pool.tile([P, 512], BF16, tag=f"qT{m}",
                                    name=f"qT{sb + 1}_{m}") for m in range(4)]
                qTs_by_sb[sb + 1] = qTs
                for m in range(4):
                    filler.append(('A', sb + 1, 0, m))
                for m in range(4):
                    filler.append(('A', sb + 1, 1, m))
                for tg in range(4):
                    filler.append(('B', sb + 1, tg))
            if sb >= 1:
                for t128 in range(4):
                    for nh in range(2):
                        filler.append(('D', sb - 1, t128, nh))

            def run_filler(frac_lo, frac_hi):
                n = len(filler)
                for u in filler[int(n * frac_lo):int(n * frac_hi)]:
                    if u[0] == 'x':
                        emit_x_dma(u[1])
                    elif u[0] == 'A':
                        emit_A_unit(u[1], u[2], u[3], qTs_by_sb[u[1]])
                    elif u[0] == 'B':
                        emit_B_unit(u[1], u[2])
                    else:
                        emit_D_unit(u[1], u[2], u[3])

            for hp in range(4):
                last_avSA = emit_C_hp(sb, hp, qTs_by_sb[sb])
                run_filler(hp / 4.0, (hp + 1) / 4.0)

        for t128 in range(4):
            for nh in range(2):
                emit_D_unit(3, t128, nh)

    nc.compile()
    return nc


def _in_maps(x, Wq, bq, Wk, bk, Wv, bv, Wp, bp):
    maskv = (np.arange(P)[:, None] <= np.arange(P)[None, :]).astype(np.float32)
    maps = []
    for c in range(8):
        b, g = divmod(c, 2)
        gs = slice(512 * g, 512 * (g + 1))
        maps.append({
            "xt": np.ascontiguousarray(x[b].T).astype(ml_dtypes.bfloat16),
            "wq": np.ascontiguousarray(Wq[gs, :].T).astype(ml_dtypes.bfloat16),
            "wk": np.ascontiguousarray(Wk[gs, :].T).astype(ml_dtypes.bfloat16),
            "wv": np.ascontiguousarray(Wv[gs, :].T).astype(ml_dtypes.bfloat16),
            "wp": np.ascontiguousarray(Wp[:, gs].T).astype(ml_dtypes.bfloat16),
            "bqk": np.stack([bq[gs].reshape(4, P).T, bk[gs].reshape(4, P).T]),
            "bvb": np.broadcast_to(bv[gs], (P, 512)).astype(ml_dtypes.bfloat16),
            "masks": maskv.astype(ml_dtypes.bfloat16),
            "ones": np.ones((P, HPC), ml_dtypes.bfloat16),
        })
    return maps


def kernel(x, Wq, bq, Wk, bk, Wv, bv, Wp, bp, _trace=False):
    if "nc" not in _CACHE:
        _CACHE["nc"] = _build()
    nc = _CACHE["nc"]
    res = run_bass_kernel_spmd(nc, _in_maps(x, Wq, bq, Wk, bk, Wv, bv, Wp, bp),
                               list(range(8)), trace=_trace)
    _CACHE["last_result"] = res
    out = np.empty((4, T, E), np.float32)
    for b in range(4):
        out[b] = res.results[2 * b]["out"] + res.results[2 * b + 1]["out"] + bp
    return out
